# revision 23
# baseline (speedup 1.0000x reference)
"""MinLSTM Trainium2 kernel — fp8 DoubleRow matmuls + fused DVE gates.

Full-input contract: kernel(**inputs) takes the complete (unsharded) numpy
inputs of the reference model and returns the full [B, T+1, H] float32 output.

Math (per batch b, channel h — identical to the reference's log-space scan,
computed in linear space; every quantity is positive so the linear recurrence
is numerically stable):
    a = x @ W_f ;  b = x @ W_i ;  c = x @ W_h          (zero biases)
    f = sigmoid(a) / (sigmoid(a) + sigmoid(b))          # forget gate
    g = max(c + 0.5, sigmoid(c))                        # = exp(log_g(c))
    h_t = f_t h_{t-1} + (1 - f_t) g_t,  h_{-1} = g(h_0)

Sharding: 8 cores, core c -> (sample b = c//2, H-half hh = c%2, 256 channels).
Fully independent cores, no collectives.

Device pipeline per core (T chunks of 512, pairs of chunks batched for the
elementwise stages):
  PE    fp8-e4m3 DoubleRow matmuls (0.5 cyc/row): a,b from x8; c from
        x8*W8h + dx8*W8h + x8*dW8h (x- and W-residual streams make the
        c-projection ~bf16-accurate; a,b tolerate plain fp8). The +0.5*beta
        bias of c rides a sacrificial constant-1 channel of the dx8 stream.
  ACT   one sigmoid pass over the [a;b] PSUM pair -> sa, sb (f32 SBUF)
  DVE   two fused custom ops:
          F:  f = sa * recip1(sa+sb)        (bitwise-NOT seed + linear poly)
          V:  vbar = (f-1) * max(m, q2(min(m,1))^2),  m = beta*(c+0.5) PSUM
  Pool  tensor_tensor_scan: state = f*state - vbar  (f32 state, f16 io)
  DMA   h~ = beta*h written as f16; host divides by beta and transposes.

beta (=1.75) rescales the c-gate so the sigma-branch quadratic fits in the
custom op's 3 scalar slots; the scan is linear in (v, init) so scaling g0 by
beta scales h exactly.
"""

from contextlib import ExitStack

import numpy as np
import ml_dtypes

import concourse.bacc as bacc
import concourse.tile as tile
import concourse.mybir as mybir
from concourse.bass_utils import run_bass_kernel_spmd

import concourse.dve_ops as _dve_ops
from concourse.dve_spec import (Spec as _Spec, Src0 as _S0, Src1 as _S1,
                                C0 as _C0, C1 as _C1, C2 as _C2, One as _One,
                                AluOp as _AluOp, Bin as _Bin, maxx as _maxx,
                                minn as _minn, lower as _lower)
from concourse.dve_uop import DveOpSpec as _DveOpSpec
from concourse.dve_table_gen import dve_ver_for as _dve_ver_for

F8 = mybir.dt.float8e4
F16 = mybir.dt.float16
F32 = mybir.dt.float32
AF = mybir.ActivationFunctionType
OP = mybir.AluOpType
DR = mybir.MatmulPerfMode.DoubleRow
NPF8 = ml_dtypes.float8_e4m3

B, T, D, H = 4, 8192, 512, 512
NCORES = 8
HS = H // 2          # 256 channels per core
TC = 512             # matmul T-chunk width
NCH = T // TC        # 16 chunks
NSUP = NCH // 2      # 8 superblocks (2 chunks each) for the DVE/scan stages
TW = 2 * TC          # superblock width
NHT = HS // 128      # 2 h-tiles per core

BETA = 1.75
# f = sa * r, r ~ 1/(sa+sb): NOT-seed nx, u = x*nx in [-4.5,-4], r=(u*c0+c1)*nx
F_CONSTS = (-0.05560890019581849, -0.4720664899356389)
# vbar = (f-1)*max(m, q(min(m,1))^2), q = (m*k0+k1)*m+k2 fits
# sqrt(beta*sigmoid(m/beta-0.5)) on m in [-6*beta, 1]
V_CONSTS = (0.007289407906601352, 0.14709027872923935, 0.802180149132902)


def _register_op(name, body, ref, rd1=True):
    if name in _dve_ops._SUB_OPCODE_FOR_NAME:
        return next(o for o in _dve_ops.OPS if o.name == name)
    spec = _Spec(body=body, reference=ref)
    row = _dve_ops._CUSTOM_DVE_ROW_BASE + len(_dve_ops.OPS)
    assert row < 0x20
    ver = _dve_ver_for("TRN2")
    sha = _DveOpSpec(name=name, opcode=row, uops=_lower(spec, ver=ver),
                     rd1_en=rd1).sha(ver)
    op = _dve_ops.DveOp(name, spec, subdim=False, uops_sha={ver: sha})
    _dve_ops.OPS.append(op)
    _dve_ops.CUSTOM_DVE_SPECS[name] = spec
    _dve_ops._SUB_OPCODE_FOR_NAME[name] = row
    return op


def _f_ref(in0, in1, c0, c1, c2):
    sa = np.asarray(in0, np.float32)
    x = (sa + np.asarray(in1, np.float32)).astype(np.float32)
    nx = (~x.view(np.int32)).view(np.float32)
    u = x * nx
    return sa * ((u * c0 + c1) * nx)


def _g_ref(in0, in1, c0, c1, c2):
    m = np.asarray(in0, np.float32)
    mc = np.minimum(m, np.float32(1.0))
    q = (mc * c0 + c1) * mc + c2
    return np.maximum(m, q * q).astype(np.float32)


def _build_f_op():
    x = _S0 + _S1
    nx = _Bin(_AluOp.BITWISE_NOT, x, x)
    u = x * nx
    body = _S0 * ((u * _C0 + _C1) * nx)
    return _register_op("MINLSTM_FGATE_ANT", body, _f_ref)


def _build_g_op():
    m = _minn(_S0, _One)
    q = (m * _C0 + _C1) * m + _C2
    body = _maxx(_S0, q * q)
    return _register_op("MINLSTM_GGATE_ANT", body, _g_ref, rd1=False)


_F_OP = _build_f_op()
_G_OP = _build_g_op()

_nc_cache = {}


def _emit_scan(nc, hpool, auxt, carry, out, item):
    t0, tw, ht, f_t, vb_t = item
    h_t = hpool.tile([128, TW], F16, tag=f"h{ht}", name="h")
    ini = auxt[:, ht:ht + 1] if t0 == 0 else carry[ht]
    nc.vector.tensor_tensor_scan(h_t[:, :tw], f_t[:, :tw], vb_t[:, :tw], ini,
                                 OP.mult, OP.add)
    carry[ht] = h_t[:, tw - 1:tw]
    nc.sync.dma_start(out[:, ht, slice(t0, t0 + tw)], h_t[:, :tw])


def _build_nc():
    nc = bacc.Bacc("TRN2", target_bir_lowering=False, debug=False,
                   num_devices=NCORES)
    # xin rows: idx = s*4 + kg*2 + i  (s: 0=x8 1=dx8; kg: k-group; i: pair)
    xin = nc.dram_tensor("xin", [128, 8, T], F8, kind="ExternalInput")
    # wab rows: idx = ((kg*2 + g)*2 + ht)*2 + i, cols m   (g: 0=W_f 1=W_i)
    wab = nc.dram_tensor("wab", [128, 16, 128], F8, kind="ExternalInput")
    # wc rows: idx = ((kg*3 + role)*2 + ht)*2 + i  (role: 0=W8h 1=Wdx 2=dWh8)
    wc = nc.dram_tensor("wc", [128, 24, 128], F8, kind="ExternalInput")
    aux = nc.dram_tensor("aux", [128, NHT], F32, kind="ExternalInput")
    out = nc.dram_tensor("out", [128, NHT, T], F16, kind="ExternalOutput")

    with tile.TileContext(nc) as tc, ExitStack() as ctx:
        wpool = ctx.enter_context(tc.tile_pool(name="w", bufs=1))
        xpool = ctx.enter_context(tc.tile_pool(name="x", bufs=6))
        spool = ctx.enter_context(tc.tile_pool(name="s", bufs=4))
        gpool = ctx.enter_context(tc.tile_pool(name="g", bufs=4))
        hpool = ctx.enter_context(tc.tile_pool(name="h", bufs=3))
        ppool = ctx.enter_context(tc.tile_pool(name="p", bufs=2, space="PSUM"))

        wab_t = wpool.tile([128, 16, 128], F8, tag="wab")
        nc.scalar.dma_start(wab_t[:], wab[:])
        wc_t = wpool.tile([128, 24, 128], F8, tag="wc")
        auxt = wpool.tile([128, NHT], F32, tag="aux")

        carry = [None] * NHT
        first_x = True
        pending = []  # (t0, tw, ht, f_t, vb_t) scans delayed one superblock
        # so the in-order DVE stream never stalls on Pool's vb multiply.
        # First/last superblocks are half-width to shorten pipeline fill/drain.
        sups = ([(0, TC)] + [(TC + TW * k, TW) for k in range(NSUP - 1)]
                + [(T - TC, TC)])
        for t0, tw in sups:
            tw2 = tw // 2
            pcs, sabs = [], []
            for ht in range(NHT):
                pcs.append(ppool.tile([128, 2, TC], F32, tag="pc", bufs=2,
                                      name=f"pc{ht}"))
                sabs.append(spool.tile([128, 2, TW], F16, tag="sab", bufs=3,
                                       name=f"sab{ht}"))
            for e in range(2):
                csl = slice(t0 + e * tw2, t0 + (e + 1) * tw2)
                xt = xpool.tile([128, 8, TC], F8, tag="x", name="x")
                nc.sync.dma_start(xt[:, :, :tw2], xin[:, :, csl])
                if first_x:
                    # wc/aux ship after the first x chunk so the a,b matmul
                    # inputs hit the DMA pipe first
                    nc.scalar.dma_start(wc_t[:], wc[:])
                    nc.scalar.dma_start(auxt[:], aux[:])
                    first_x = False

                def rhs(s, kg):
                    r0 = s * 4 + kg * 2
                    return xt[:, r0:r0 + 2, :tw2]

                for ht in range(NHT):
                    pfi = ppool.tile([128, 2, TC], F32, tag="pfi", bufs=2)
                    for g in range(2):
                        for kg in range(2):
                            w0 = ((kg * 2 + g) * 2 + ht) * 2
                            nc.tensor.matmul(
                                pfi[:, g, :tw2], wab_t[:, w0:w0 + 2, :],
                                rhs(0, kg), start=(kg == 0), stop=(kg == 1),
                                perf_mode=DR)
                    cparts = [(0, 0), (1, 1), (2, 0)]  # (role, stream)
                    for pi, (role, s) in enumerate(cparts):
                        for kg in range(2):
                            w0 = ((kg * 3 + role) * 2 + ht) * 2
                            nc.tensor.matmul(
                                pcs[ht][:, e, :tw2], wc_t[:, w0:w0 + 2, :],
                                rhs(s, kg),
                                start=(pi == 0 and kg == 0),
                                stop=(pi == 2 and kg == 1), perf_mode=DR)
                    nc.scalar.activation(
                        sabs[ht][:, :, e * tw2:(e + 1) * tw2],
                        pfi[:, :, :tw2], AF.Sigmoid)
            nxt = []
            for ht in range(NHT):
                # f = sa * ~1/(sa+sb) in one fused custom DVE op
                f_t = gpool.tile([128, TW], F16, tag=f"f{ht}", name="f")
                nc.vector._custom_dve(_F_OP, out=f_t[:, :tw],
                                      in0=sabs[ht][:, 0, :tw],
                                      in1=sabs[ht][:, 1, :tw],
                                      s0=F_CONSTS[0], s1=F_CONSTS[1])
                g_t = gpool.tile([128, TW], F16, tag=f"g{ht}", name="g")
                nc.vector._custom_dve(_G_OP, out=g_t[:, :tw],
                                      in0=pcs[ht][:, :, :tw2],
                                      s0=V_CONSTS[0], s1=V_CONSTS[1],
                                      imm2=V_CONSTS[2])
                w_t = gpool.tile([128, TW], F16, tag=f"w{ht}", name="w")
                nc.scalar.activation(w_t[:, :tw], f_t[:, :tw], AF.Copy,
                                     bias=1.0, scale=-1.0)
                vb_t = gpool.tile([128, TW], F16, tag=f"v{ht}", name="v")
                nc.gpsimd.tensor_tensor(vb_t[:, :tw], w_t[:, :tw],
                                        g_t[:, :tw], op=OP.mult)
                nxt.append((t0, tw, ht, f_t, vb_t))
            for item in pending:
                _emit_scan(nc, hpool, auxt, carry, out, item)
            pending = nxt
        for item in pending:
            _emit_scan(nc, hpool, auxt, carry, out, item)
    nc.compile()
    return nc


def _get_nc():
    if "nc" not in _nc_cache:
        _nc_cache["nc"] = _build_nc()
    return _nc_cache["nc"]


def _g_host(x):
    # exp(log_g(x)) of the reference, computed directly in fp32
    return np.where(x >= 0, x + 0.5, 1.0 / (1.0 + np.exp(-np.minimum(x, 0))))


def _pack_dpairs(mat):
    """[T-or-D rows laid out d, cols] -> rows reindexed (kg, k, i): d = kg*256 + 2k + i.
    Input [D, N] -> output [128, 2kg, 2i, N] with out[k, kg, i] = in[kg*256+2k+i]."""
    m = mat.reshape(2, 128, 2, -1)          # [kg, k, i, N]
    return np.ascontiguousarray(m.transpose(1, 0, 2, 3))  # [k, kg, i, N]


def _run(inputs, trace=False):
    x = np.asarray(inputs["x"], np.float32)
    h_0 = np.asarray(inputs["h_0"], np.float32)
    W = {k: np.asarray(inputs[k], np.float32)
         for k in ("W_f", "W_i", "W_h")}
    for k in ("b_f", "b_i", "b_h"):
        assert (np.asarray(inputs[k]) == 0).all(), \
            "device program folds zero biases"

    g0 = _g_host(h_0[:, 0, :])  # [B, H]

    # --- x streams, packed once per sample ------------------------------
    xins = []
    for b in range(B):
        x8 = x[b].astype(NPF8)                       # [T, D]
        dx = (x[b] - x8.astype(np.float32)).astype(NPF8)
        # [2s, T, D] -> rows (k, s, kg, i) cols T
        s_td = np.stack([x8, dx], axis=0)            # [2, T, D]
        m = s_td.reshape(2, T, 2, 128, 2)            # [s, T, kg, k, i]
        m = m.transpose(3, 0, 2, 4, 1)               # [k, s, kg, i, T]
        m = np.ascontiguousarray(m).reshape(128, 8, T)
        m[127, 7, :] = NPF8(1.0)                     # bias slot: s=1,kg=1,i=1
        xins.append(m)

    in_maps = []
    for c in range(NCORES):
        b, hh = divmod(c, 2)
        hs = slice(hh * HS, (hh + 1) * HS)
        w8f = _pack_dpairs(W["W_f"][:, hs].astype(NPF8))   # [128,2,2,256]
        w8i = _pack_dpairs(W["W_i"][:, hs].astype(NPF8))
        whb = BETA * W["W_h"][:, hs]
        w8h = whb.astype(NPF8)
        dwh = (whb - w8h.astype(np.float32)).astype(NPF8)
        w8h_p = _pack_dpairs(w8h)
        dwh_p = _pack_dpairs(dwh)
        wdx_p = w8h_p.copy()
        wdx_p[127, 1, 1, :] = NPF8(0.5 * BETA)       # bias row (d=511)
        # wab rows: ((kg*2+g)*2+ht)*2+i ; cols m in [0,128)
        wab = np.zeros((128, 16, 128), NPF8)
        wcm = np.zeros((128, 24, 128), NPF8)
        for kg in range(2):
            for i in range(2):
                for ht in range(NHT):
                    mcols = slice(ht * 128, (ht + 1) * 128)
                    for g, wp in ((0, w8f), (1, w8i)):
                        wab[:, ((kg * 2 + g) * 2 + ht) * 2 + i, :] = \
                            wp[:, kg, i, mcols]
                    for role, wp in ((0, w8h_p), (1, wdx_p), (2, dwh_p)):
                        wcm[:, ((kg * 3 + role) * 2 + ht) * 2 + i, :] = \
                            wp[:, kg, i, mcols]
        auxa = np.ascontiguousarray(
            (BETA * g0[b, hs]).reshape(NHT, 128).T.astype(np.float32))
        in_maps.append({"xin": xins[b], "wab": wab, "wc": wcm, "aux": auxa})

    nc = _get_nc()
    res = run_bass_kernel_spmd(nc, in_maps, core_ids=list(range(NCORES)),
                               trace=trace)

    out = np.empty((B, T + 1, H), np.float32)
    out[:, 0, :] = g0
    inv_beta = np.float32(1.0 / BETA)
    for c in range(NCORES):
        b, hh = divmod(c, 2)
        o = np.asarray(res.results[c]["out"], np.float32)  # [128, NHT, T]
        # channel = hh*256 + ht*128 + p
        out[b, 1:, hh * HS:(hh + 1) * HS] = \
            (o.transpose(2, 1, 0).reshape(T, HS)) * inv_beta
    return out, res


def kernel(**inputs):
    out, _ = _run(inputs)
    return out


# revision 30
# speedup vs baseline: 1.0174x; 1.0174x over previous
"""MinLSTM Trainium2 kernel — fp8 DoubleRow matmuls + fused DVE gates.

Full-input contract: kernel(**inputs) takes the complete (unsharded) numpy
inputs of the reference model and returns the full [B, T+1, H] float32 output.

Math (per batch b, channel h — identical to the reference's log-space scan,
computed in linear space; every quantity is positive so the linear recurrence
is numerically stable):
    a = x @ W_f ;  b = x @ W_i ;  c = x @ W_h          (zero biases)
    f = sigmoid(a) / (sigmoid(a) + sigmoid(b))          # forget gate
    g = max(c + 0.5, sigmoid(c))                        # = exp(log_g(c))
    h_t = f_t h_{t-1} + (1 - f_t) g_t,  h_{-1} = g(h_0)

Sharding: 8 cores, core c -> (sample b = c//2, H-half hh = c%2, 256 channels).
Fully independent cores, no collectives.

Device pipeline per core (T chunks of 512, pairs of chunks batched for the
elementwise stages):
  PE    fp8-e4m3 DoubleRow matmuls (0.5 cyc/row): a,b from x8; c from
        x8*W8h + dx8*W8h + x8*dW8h (x- and W-residual streams make the
        c-projection ~bf16-accurate; a,b tolerate plain fp8). The +0.5*beta
        bias of c rides a sacrificial constant-1 channel of the dx8 stream.
  ACT   one sigmoid pass over the [a;b] PSUM pair -> sa, sb (f32 SBUF)
  DVE   two fused custom ops:
          F:  f = sa * recip1(sa+sb)        (bitwise-NOT seed + linear poly)
          V:  vbar = (f-1) * max(m, q2(min(m,1))^2),  m = beta*(c+0.5) PSUM
  Pool  tensor_tensor_scan: state = f*state - vbar  (f32 state, f16 io)
  DMA   h~ = beta*h written as f16; host divides by beta and transposes.

beta (=1.75) rescales the c-gate so the sigma-branch quadratic fits in the
custom op's 3 scalar slots; the scan is linear in (v, init) so scaling g0 by
beta scales h exactly.
"""

from contextlib import ExitStack

import numpy as np
import ml_dtypes

import concourse.bacc as bacc
import concourse.tile as tile
import concourse.mybir as mybir
from concourse.bass_utils import run_bass_kernel_spmd

import concourse.dve_ops as _dve_ops
from concourse.dve_spec import (Spec as _Spec, Src0 as _S0, Src1 as _S1,
                                C0 as _C0, C1 as _C1, C2 as _C2, One as _One,
                                AluOp as _AluOp, Bin as _Bin, maxx as _maxx,
                                minn as _minn, lower as _lower)
from concourse.dve_uop import DveOpSpec as _DveOpSpec
from concourse.dve_table_gen import dve_ver_for as _dve_ver_for

F8 = mybir.dt.float8e4
F16 = mybir.dt.float16
F32 = mybir.dt.float32
AF = mybir.ActivationFunctionType
OP = mybir.AluOpType
DR = mybir.MatmulPerfMode.DoubleRow
NPF8 = ml_dtypes.float8_e4m3

B, T, D, H = 4, 8192, 512, 512
NCORES = 8
HS = H // 2          # 256 channels per core
TC = 512             # matmul T-chunk width
NCH = T // TC        # 16 chunks
NSUP = NCH // 2      # 8 superblocks (2 chunks each) for the DVE/scan stages
TW = 2 * TC          # superblock width
NHT = HS // 128      # 2 h-tiles per core

BETA = 1.75
# f = sa * r, r ~ 1/(sa+sb): NOT-seed nx, u = x*nx in [-4.5,-4], r=(u*c0+c1)*nx
F_CONSTS = (-0.05560890019581849, -0.4720664899356389)
# vbar = (f-1)*max(m, q(min(m,1))^2), q = (m*k0+k1)*m+k2 fits
# sqrt(beta*sigmoid(m/beta-0.5)) on m in [-6*beta, 1]
V_CONSTS = (0.007289407906601352, 0.14709027872923935, 0.802180149132902)


def _register_op(name, body, ref, rd1=True):
    if name in _dve_ops._SUB_OPCODE_FOR_NAME:
        return next(o for o in _dve_ops.OPS if o.name == name)
    spec = _Spec(body=body, reference=ref)
    row = _dve_ops._CUSTOM_DVE_ROW_BASE + len(_dve_ops.OPS)
    assert row < 0x20
    ver = _dve_ver_for("TRN2")
    sha = _DveOpSpec(name=name, opcode=row, uops=_lower(spec, ver=ver),
                     rd1_en=rd1).sha(ver)
    op = _dve_ops.DveOp(name, spec, subdim=False, uops_sha={ver: sha})
    _dve_ops.OPS.append(op)
    _dve_ops.CUSTOM_DVE_SPECS[name] = spec
    _dve_ops._SUB_OPCODE_FOR_NAME[name] = row
    return op


def _f_ref(in0, in1, c0, c1, c2):
    sa = np.asarray(in0, np.float32)
    x = (sa + np.asarray(in1, np.float32)).astype(np.float32)
    nx = (~x.view(np.int32)).view(np.float32)
    u = x * nx
    return sa * ((u * c0 + c1) * nx)


def _g_ref(in0, in1, c0, c1, c2):
    m = np.asarray(in0, np.float32)
    mc = np.minimum(m, np.float32(1.0))
    q = (mc * c0 + c1) * mc + c2
    return np.maximum(m, q * q).astype(np.float32)


def _build_f_op():
    x = _S0 + _S1
    nx = _Bin(_AluOp.BITWISE_NOT, x, x)
    u = x * nx
    body = _S0 * ((u * _C0 + _C1) * nx)
    return _register_op("MINLSTM_FGATE_ANT", body, _f_ref)


def _build_g_op():
    m = _minn(_S0, _One)
    q = (m * _C0 + _C1) * m + _C2
    body = _maxx(_S0, q * q)
    return _register_op("MINLSTM_GGATE_ANT", body, _g_ref, rd1=False)


_F_OP = _build_f_op()
_G_OP = _build_g_op()

_nc_cache = {}


def _emit_scan(nc, hpool, auxt, carry, out, item):
    t0, tw, ht, f_t, vb_t = item
    h_t = hpool.tile([128, TW], F16, tag=f"h{ht}", name="h")
    ini = auxt[:, ht:ht + 1] if t0 == 0 else carry[ht]
    nc.vector.tensor_tensor_scan(h_t[:, :tw], f_t[:, :tw], vb_t[:, :tw], ini,
                                 OP.mult, OP.add)
    carry[ht] = h_t[:, tw - 1:tw]
    nc.sync.dma_start(out[:, ht, slice(t0, t0 + tw)], h_t[:, :tw])


def _build_nc():
    nc = bacc.Bacc("TRN2", target_bir_lowering=False, debug=False,
                   num_devices=NCORES)
    # xin rows: idx = s*4 + kg*2 + i  (s: 0=x8 1=dx8; kg: k-group; i: pair)
    xin = nc.dram_tensor("xin", [128, 8, T], F8, kind="ExternalInput")
    # wab rows: idx = ((kg*2 + g)*2 + ht)*2 + i, cols m   (g: 0=W_f 1=W_i)
    wab = nc.dram_tensor("wab", [128, 16, 128], F8, kind="ExternalInput")
    # wc rows: idx = ((kg*3 + role)*2 + ht)*2 + i  (role: 0=W8h 1=Wdx 2=dWh8)
    wc = nc.dram_tensor("wc", [128, 24, 128], F8, kind="ExternalInput")
    aux = nc.dram_tensor("aux", [128, NHT], F32, kind="ExternalInput")
    out = nc.dram_tensor("out", [128, NHT, T], F16, kind="ExternalOutput")

    with tile.TileContext(nc) as tc, ExitStack() as ctx:
        wpool = ctx.enter_context(tc.tile_pool(name="w", bufs=1))
        xpool = ctx.enter_context(tc.tile_pool(name="x", bufs=6))
        spool = ctx.enter_context(tc.tile_pool(name="s", bufs=4))
        gpool = ctx.enter_context(tc.tile_pool(name="g", bufs=4))
        hpool = ctx.enter_context(tc.tile_pool(name="h", bufs=3))
        ppool = ctx.enter_context(tc.tile_pool(name="p", bufs=2, space="PSUM"))

        wab_t = wpool.tile([128, 16, 128], F8, tag="wab")
        nc.scalar.dma_start(wab_t[:], wab[:])
        wc_t = wpool.tile([128, 24, 128], F8, tag="wc")
        auxt = wpool.tile([128, NHT], F32, tag="aux")
        # tiny dummy activations force the ACT table loads to happen before
        # the data DMAs / matmuls fill the pipe (the implicit LoadActFuncSet
        # otherwise lands right before the first real sigmoid)
        scr = wpool.tile([128, 1], F32, tag="scr")
        nc.gpsimd.memset(scr[:], 0.0)
        scr2 = wpool.tile([128, 1], F32, tag="scr2")
        nc.scalar.activation(scr2[:], scr[:], AF.Sigmoid)
        nc.scalar.activation(scr2[:], scr[:], AF.Copy, bias=1.0, scale=-1.0)

        carry = [None] * NHT
        first_x = True
        pending = []  # (t0, tw, ht, f_t, vb_t) scans delayed one superblock
        # so the in-order DVE stream never stalls on Pool's vb multiply.
        # First/last superblocks are half-width to shorten pipeline fill/drain.
        sups = ([(0, TC)] + [(TC + TW * k, TW) for k in range(NSUP - 1)]
                + [(T - TC, TC)])
        for t0, tw in sups:
            tw2 = tw // 2
            pcs, sabs = [], []
            for ht in range(NHT):
                pcs.append(ppool.tile([128, 2, TC], F32, tag="pc", bufs=2,
                                      name=f"pc{ht}"))
                sabs.append(spool.tile([128, 2, TW], F16, tag="sab", bufs=3,
                                       name=f"sab{ht}"))
            for e in range(2):
                csl = slice(t0 + e * tw2, t0 + (e + 1) * tw2)
                xt = xpool.tile([128, 8, TC], F8, tag="x", name="x")
                nc.sync.dma_start(xt[:, :, :tw2], xin[:, :, csl])
                if first_x:
                    # wc/aux ship after the first x chunk so the a,b matmul
                    # inputs hit the DMA pipe first
                    nc.scalar.dma_start(wc_t[:], wc[:])
                    nc.scalar.dma_start(auxt[:], aux[:])
                    first_x = False

                def rhs(s, kg):
                    r0 = s * 4 + kg * 2
                    return xt[:, r0:r0 + 2, :tw2]

                for ht in range(NHT):
                    pfi = ppool.tile([128, 2, TC], F32, tag="pfi", bufs=2)
                    for g in range(2):
                        for kg in range(2):
                            w0 = ((kg * 2 + g) * 2 + ht) * 2
                            nc.tensor.matmul(
                                pfi[:, g, :tw2], wab_t[:, w0:w0 + 2, :],
                                rhs(0, kg), start=(kg == 0), stop=(kg == 1),
                                perf_mode=DR)
                    cparts = [(0, 0), (1, 1), (2, 0)]  # (role, stream)
                    for pi, (role, s) in enumerate(cparts):
                        for kg in range(2):
                            w0 = ((kg * 3 + role) * 2 + ht) * 2
                            nc.tensor.matmul(
                                pcs[ht][:, e, :tw2], wc_t[:, w0:w0 + 2, :],
                                rhs(s, kg),
                                start=(pi == 0 and kg == 0),
                                stop=(pi == 2 and kg == 1), perf_mode=DR)
                    nc.scalar.activation(
                        sabs[ht][:, :, e * tw2:(e + 1) * tw2],
                        pfi[:, :, :tw2], AF.Sigmoid)
            nxt = []
            for ht in range(NHT):
                # f = sa * ~1/(sa+sb) in one fused custom DVE op
                f_t = gpool.tile([128, TW], F16, tag=f"f{ht}", name="f")
                nc.vector._custom_dve(_F_OP, out=f_t[:, :tw],
                                      in0=sabs[ht][:, 0, :tw],
                                      in1=sabs[ht][:, 1, :tw],
                                      s0=F_CONSTS[0], s1=F_CONSTS[1])
                g_t = gpool.tile([128, TW], F16, tag=f"g{ht}", name="g")
                nc.vector._custom_dve(_G_OP, out=g_t[:, :tw],
                                      in0=pcs[ht][:, :, :tw2],
                                      s0=V_CONSTS[0], s1=V_CONSTS[1],
                                      imm2=V_CONSTS[2])
                w_t = gpool.tile([128, TW], F16, tag=f"w{ht}", name="w")
                nc.scalar.activation(w_t[:, :tw], f_t[:, :tw], AF.Copy,
                                     bias=1.0, scale=-1.0)
                vb_t = gpool.tile([128, TW], F16, tag=f"v{ht}", name="v")
                nc.gpsimd.tensor_tensor(vb_t[:, :tw], w_t[:, :tw],
                                        g_t[:, :tw], op=OP.mult)
                nxt.append((t0, tw, ht, f_t, vb_t))
            for item in pending:
                _emit_scan(nc, hpool, auxt, carry, out, item)
            pending = nxt
        for item in pending:
            _emit_scan(nc, hpool, auxt, carry, out, item)
    nc.compile()
    return nc


def _get_nc():
    if "nc" not in _nc_cache:
        _nc_cache["nc"] = _build_nc()
    return _nc_cache["nc"]


def _g_host(x):
    # exp(log_g(x)) of the reference, computed directly in fp32
    return np.where(x >= 0, x + 0.5, 1.0 / (1.0 + np.exp(-np.minimum(x, 0))))


def _pack_dpairs(mat):
    """[T-or-D rows laid out d, cols] -> rows reindexed (kg, k, i): d = kg*256 + 2k + i.
    Input [D, N] -> output [128, 2kg, 2i, N] with out[k, kg, i] = in[kg*256+2k+i]."""
    m = mat.reshape(2, 128, 2, -1)          # [kg, k, i, N]
    return np.ascontiguousarray(m.transpose(1, 0, 2, 3))  # [k, kg, i, N]


def _run(inputs, trace=False):
    x = np.asarray(inputs["x"], np.float32)
    h_0 = np.asarray(inputs["h_0"], np.float32)
    W = {k: np.asarray(inputs[k], np.float32)
         for k in ("W_f", "W_i", "W_h")}
    for k in ("b_f", "b_i", "b_h"):
        assert (np.asarray(inputs[k]) == 0).all(), \
            "device program folds zero biases"

    g0 = _g_host(h_0[:, 0, :])  # [B, H]

    # --- x streams, packed once per sample ------------------------------
    xins = []
    for b in range(B):
        x8 = x[b].astype(NPF8)                       # [T, D]
        dx = (x[b] - x8.astype(np.float32)).astype(NPF8)
        # [2s, T, D] -> rows (k, s, kg, i) cols T
        s_td = np.stack([x8, dx], axis=0)            # [2, T, D]
        m = s_td.reshape(2, T, 2, 128, 2)            # [s, T, kg, k, i]
        m = m.transpose(3, 0, 2, 4, 1)               # [k, s, kg, i, T]
        m = np.ascontiguousarray(m).reshape(128, 8, T)
        m[127, 7, :] = NPF8(1.0)                     # bias slot: s=1,kg=1,i=1
        xins.append(m)

    in_maps = []
    for c in range(NCORES):
        b, hh = divmod(c, 2)
        hs = slice(hh * HS, (hh + 1) * HS)
        w8f = _pack_dpairs(W["W_f"][:, hs].astype(NPF8))   # [128,2,2,256]
        w8i = _pack_dpairs(W["W_i"][:, hs].astype(NPF8))
        whb = BETA * W["W_h"][:, hs]
        w8h = whb.astype(NPF8)
        dwh = (whb - w8h.astype(np.float32)).astype(NPF8)
        w8h_p = _pack_dpairs(w8h)
        dwh_p = _pack_dpairs(dwh)
        wdx_p = w8h_p.copy()
        wdx_p[127, 1, 1, :] = NPF8(0.5 * BETA)       # bias row (d=511)
        # wab rows: ((kg*2+g)*2+ht)*2+i ; cols m in [0,128)
        wab = np.zeros((128, 16, 128), NPF8)
        wcm = np.zeros((128, 24, 128), NPF8)
        for kg in range(2):
            for i in range(2):
                for ht in range(NHT):
                    mcols = slice(ht * 128, (ht + 1) * 128)
                    for g, wp in ((0, w8f), (1, w8i)):
                        wab[:, ((kg * 2 + g) * 2 + ht) * 2 + i, :] = \
                            wp[:, kg, i, mcols]
                    for role, wp in ((0, w8h_p), (1, wdx_p), (2, dwh_p)):
                        wcm[:, ((kg * 3 + role) * 2 + ht) * 2 + i, :] = \
                            wp[:, kg, i, mcols]
        auxa = np.ascontiguousarray(
            (BETA * g0[b, hs]).reshape(NHT, 128).T.astype(np.float32))
        in_maps.append({"xin": xins[b], "wab": wab, "wc": wcm, "aux": auxa})

    nc = _get_nc()
    res = run_bass_kernel_spmd(nc, in_maps, core_ids=list(range(NCORES)),
                               trace=trace)

    out = np.empty((B, T + 1, H), np.float32)
    out[:, 0, :] = g0
    inv_beta = np.float32(1.0 / BETA)
    for c in range(NCORES):
        b, hh = divmod(c, 2)
        o = np.asarray(res.results[c]["out"], np.float32)  # [128, NHT, T]
        # channel = hh*256 + ht*128 + p
        out[b, 1:, hh * HS:(hh + 1) * HS] = \
            (o.transpose(2, 1, 0).reshape(T, HS)) * inv_beta
    return out, res


def kernel(**inputs):
    out, _ = _run(inputs)
    return out


# revision 31
# speedup vs baseline: 1.0273x; 1.0098x over previous
"""MinLSTM Trainium2 kernel — fp8 DoubleRow matmuls + fused DVE gates.

Full-input contract: kernel(**inputs) takes the complete (unsharded) numpy
inputs of the reference model and returns the full [B, T+1, H] float32 output.

Math (per batch b, channel h — identical to the reference's log-space scan,
computed in linear space; every quantity is positive so the linear recurrence
is numerically stable):
    a = x @ W_f ;  b = x @ W_i ;  c = x @ W_h          (zero biases)
    f = sigmoid(a) / (sigmoid(a) + sigmoid(b))          # forget gate
    g = max(c + 0.5, sigmoid(c))                        # = exp(log_g(c))
    h_t = f_t h_{t-1} + (1 - f_t) g_t,  h_{-1} = g(h_0)

Sharding: 8 cores, core c -> (sample b = c//2, H-half hh = c%2, 256 channels).
Fully independent cores, no collectives.

Device pipeline per core (T chunks of 512, pairs of chunks batched for the
elementwise stages):
  PE    fp8-e4m3 DoubleRow matmuls (0.5 cyc/row): a,b from x8; c from
        x8*W8h + dx8*W8h + x8*dW8h (x- and W-residual streams make the
        c-projection ~bf16-accurate; a,b tolerate plain fp8). The +0.5*beta
        bias of c rides a sacrificial constant-1 channel of the dx8 stream.
  ACT   one sigmoid pass over the [a;b] PSUM pair -> sa, sb (f32 SBUF)
  DVE   two fused custom ops:
          F:  f = sa * recip1(sa+sb)        (bitwise-NOT seed + linear poly)
          V:  vbar = (f-1) * max(m, q2(min(m,1))^2),  m = beta*(c+0.5) PSUM
  Pool  tensor_tensor_scan: state = f*state - vbar  (f32 state, f16 io)
  DMA   h~ = beta*h written as f16; host divides by beta and transposes.

beta (=1.75) rescales the c-gate so the sigma-branch quadratic fits in the
custom op's 3 scalar slots; the scan is linear in (v, init) so scaling g0 by
beta scales h exactly.
"""

from contextlib import ExitStack

import numpy as np
import ml_dtypes

import concourse.bacc as bacc
import concourse.tile as tile
import concourse.mybir as mybir
from concourse.bass_utils import run_bass_kernel_spmd

import concourse.dve_ops as _dve_ops
from concourse.dve_spec import (Spec as _Spec, Src0 as _S0, Src1 as _S1,
                                C0 as _C0, C1 as _C1, C2 as _C2, One as _One,
                                AluOp as _AluOp, Bin as _Bin, maxx as _maxx,
                                minn as _minn, lower as _lower)
from concourse.dve_uop import DveOpSpec as _DveOpSpec
from concourse.dve_table_gen import dve_ver_for as _dve_ver_for

F8 = mybir.dt.float8e4
F16 = mybir.dt.float16
F32 = mybir.dt.float32
AF = mybir.ActivationFunctionType
OP = mybir.AluOpType
DR = mybir.MatmulPerfMode.DoubleRow
NPF8 = ml_dtypes.float8_e4m3

B, T, D, H = 4, 8192, 512, 512
NCORES = 8
HS = H // 2          # 256 channels per core
TC = 512             # matmul T-chunk width
NCH = T // TC        # 16 chunks
NSUP = NCH // 2      # 8 superblocks (2 chunks each) for the DVE/scan stages
TW = 2 * TC          # superblock width
NHT = HS // 128      # 2 h-tiles per core

BETA = 1.75
# f = sa * r, r ~ 1/(sa+sb): NOT-seed nx, u = x*nx in [-4.5,-4], r=(u*c0+c1)*nx
F_CONSTS = (-0.05560890019581849, -0.4720664899356389)
# vbar = (f-1)*max(m, q(min(m,1))^2), q = (m*k0+k1)*m+k2 fits
# sqrt(beta*sigmoid(m/beta-0.5)) on m in [-6*beta, 1]
V_CONSTS = (0.007289407906601352, 0.14709027872923935, 0.802180149132902)


def _register_op(name, body, ref, rd1=True):
    if name in _dve_ops._SUB_OPCODE_FOR_NAME:
        return next(o for o in _dve_ops.OPS if o.name == name)
    spec = _Spec(body=body, reference=ref)
    row = _dve_ops._CUSTOM_DVE_ROW_BASE + len(_dve_ops.OPS)
    assert row < 0x20
    ver = _dve_ver_for("TRN2")
    sha = _DveOpSpec(name=name, opcode=row, uops=_lower(spec, ver=ver),
                     rd1_en=rd1).sha(ver)
    op = _dve_ops.DveOp(name, spec, subdim=False, uops_sha={ver: sha})
    _dve_ops.OPS.append(op)
    _dve_ops.CUSTOM_DVE_SPECS[name] = spec
    _dve_ops._SUB_OPCODE_FOR_NAME[name] = row
    return op


def _f_ref(in0, in1, c0, c1, c2):
    sa = np.asarray(in0, np.float32)
    x = (sa + np.asarray(in1, np.float32)).astype(np.float32)
    nx = (~x.view(np.int32)).view(np.float32)
    u = x * nx
    return sa * ((u * c0 + c1) * nx)


def _g_ref(in0, in1, c0, c1, c2):
    m = np.asarray(in0, np.float32)
    mc = np.minimum(m, np.float32(1.0))
    q = (mc * c0 + c1) * mc + c2
    return np.maximum(m, q * q).astype(np.float32)


def _build_f_op():
    x = _S0 + _S1
    nx = _Bin(_AluOp.BITWISE_NOT, x, x)
    u = x * nx
    body = _S0 * ((u * _C0 + _C1) * nx)
    return _register_op("MINLSTM_FGATE_ANT", body, _f_ref)


def _build_g_op():
    m = _minn(_S0, _One)
    q = (m * _C0 + _C1) * m + _C2
    body = _maxx(_S0, q * q)
    return _register_op("MINLSTM_GGATE_ANT", body, _g_ref, rd1=False)


_F_OP = _build_f_op()
_G_OP = _build_g_op()

_nc_cache = {}


def _emit_scan(nc, hpool, auxt, carry, out, item):
    t0, tw, ht, f_t, vb_t = item
    h_t = hpool.tile([128, TW], F16, tag=f"h{ht}", name="h")
    ini = auxt[:, ht:ht + 1] if t0 == 0 else carry[ht]
    nc.vector.tensor_tensor_scan(h_t[:, :tw], f_t[:, :tw], vb_t[:, :tw], ini,
                                 OP.mult, OP.add)
    carry[ht] = h_t[:, tw - 1:tw]
    nc.sync.dma_start(out[:, ht, slice(t0, t0 + tw)], h_t[:, :tw])


def _build_nc():
    nc = bacc.Bacc("TRN2", target_bir_lowering=False, debug=False,
                   num_devices=NCORES)
    # xin rows: idx = s*4 + kg*2 + i  (s: 0=x8 1=dx8; kg: k-group; i: pair)
    xin = nc.dram_tensor("xin", [128, 8, T], F8, kind="ExternalInput")
    # wab rows: idx = ((kg*2 + g)*2 + ht)*2 + i, cols m   (g: 0=W_f 1=W_i)
    wab = nc.dram_tensor("wab", [128, 16, 128], F8, kind="ExternalInput")
    # wc rows: idx = ((kg*3 + role)*2 + ht)*2 + i  (role: 0=W8h 1=Wdx 2=dWh8)
    wc = nc.dram_tensor("wc", [128, 24, 128], F8, kind="ExternalInput")
    aux = nc.dram_tensor("aux", [128, NHT], F32, kind="ExternalInput")
    out = nc.dram_tensor("out", [128, NHT, T], F16, kind="ExternalOutput")

    with tile.TileContext(nc) as tc, ExitStack() as ctx:
        wpool = ctx.enter_context(tc.tile_pool(name="w", bufs=1))
        xpool = ctx.enter_context(tc.tile_pool(name="x", bufs=6))
        spool = ctx.enter_context(tc.tile_pool(name="s", bufs=4))
        gpool = ctx.enter_context(tc.tile_pool(name="g", bufs=4))
        hpool = ctx.enter_context(tc.tile_pool(name="h", bufs=3))
        ppool = ctx.enter_context(tc.tile_pool(name="p", bufs=2, space="PSUM"))

        wab_t = wpool.tile([128, 16, 128], F8, tag="wab")
        nc.scalar.dma_start(wab_t[:], wab[:])
        wc_t = wpool.tile([128, 24, 128], F8, tag="wc")
        auxt = wpool.tile([128, NHT], F32, tag="aux")
        # tiny dummy activations force the ACT table loads to happen before
        # the data DMAs / matmuls fill the pipe (the implicit LoadActFuncSet
        # otherwise lands right before the first real sigmoid)
        scr = wpool.tile([128, 1], F32, tag="scr")
        nc.gpsimd.memset(scr[:], 0.0)
        scr2 = wpool.tile([128, 1], F32, tag="scr2")
        nc.scalar.activation(scr2[:], scr[:], AF.Sigmoid)
        nc.scalar.activation(scr2[:], scr[:], AF.Copy, bias=1.0, scale=-1.0)

        carry = [None] * NHT
        first_x = True
        pending = []  # (t0, tw, ht, f_t, vb_t) scans delayed one superblock
        # so the in-order DVE stream never stalls on Pool's vb multiply.
        # First/last superblocks are half-width to shorten pipeline fill/drain.
        sups = ([(0, 256), (256, 256)]
                + [(TC + TW * k, TW) for k in range(NSUP - 1)]
                + [(T - TC, 256), (T - 256, 256)])
        for t0, tw in sups:
            tw2 = tw // 2
            pcs, sabs = [], []
            for ht in range(NHT):
                pcs.append(ppool.tile([128, 2, TC], F32, tag="pc", bufs=2,
                                      name=f"pc{ht}"))
                sabs.append(spool.tile([128, 2, TW], F16, tag="sab", bufs=3,
                                       name=f"sab{ht}"))
            for e in range(2):
                csl = slice(t0 + e * tw2, t0 + (e + 1) * tw2)
                xt = xpool.tile([128, 8, TC], F8, tag="x", name="x")
                nc.sync.dma_start(xt[:, :, :tw2], xin[:, :, csl])
                if first_x:
                    # wc/aux ship after the first x chunk so the a,b matmul
                    # inputs hit the DMA pipe first
                    nc.scalar.dma_start(wc_t[:], wc[:])
                    nc.scalar.dma_start(auxt[:], aux[:])
                    first_x = False

                def rhs(s, kg):
                    r0 = s * 4 + kg * 2
                    return xt[:, r0:r0 + 2, :tw2]

                for ht in range(NHT):
                    pfi = ppool.tile([128, 2, TC], F32, tag="pfi", bufs=2)
                    for g in range(2):
                        for kg in range(2):
                            w0 = ((kg * 2 + g) * 2 + ht) * 2
                            nc.tensor.matmul(
                                pfi[:, g, :tw2], wab_t[:, w0:w0 + 2, :],
                                rhs(0, kg), start=(kg == 0), stop=(kg == 1),
                                perf_mode=DR)
                    cparts = [(0, 0), (1, 1), (2, 0)]  # (role, stream)
                    for pi, (role, s) in enumerate(cparts):
                        for kg in range(2):
                            w0 = ((kg * 3 + role) * 2 + ht) * 2
                            nc.tensor.matmul(
                                pcs[ht][:, e, :tw2], wc_t[:, w0:w0 + 2, :],
                                rhs(s, kg),
                                start=(pi == 0 and kg == 0),
                                stop=(pi == 2 and kg == 1), perf_mode=DR)
                    nc.scalar.activation(
                        sabs[ht][:, :, e * tw2:(e + 1) * tw2],
                        pfi[:, :, :tw2], AF.Sigmoid)
            nxt = []
            for ht in range(NHT):
                # f = sa * ~1/(sa+sb) in one fused custom DVE op
                f_t = gpool.tile([128, TW], F16, tag=f"f{ht}", name="f")
                nc.vector._custom_dve(_F_OP, out=f_t[:, :tw],
                                      in0=sabs[ht][:, 0, :tw],
                                      in1=sabs[ht][:, 1, :tw],
                                      s0=F_CONSTS[0], s1=F_CONSTS[1])
                g_t = gpool.tile([128, TW], F16, tag=f"g{ht}", name="g")
                nc.vector._custom_dve(_G_OP, out=g_t[:, :tw],
                                      in0=pcs[ht][:, :, :tw2],
                                      s0=V_CONSTS[0], s1=V_CONSTS[1],
                                      imm2=V_CONSTS[2])
                w_t = gpool.tile([128, TW], F16, tag=f"w{ht}", name="w")
                nc.scalar.activation(w_t[:, :tw], f_t[:, :tw], AF.Copy,
                                     bias=1.0, scale=-1.0)
                vb_t = gpool.tile([128, TW], F16, tag=f"v{ht}", name="v")
                nc.gpsimd.tensor_tensor(vb_t[:, :tw], w_t[:, :tw],
                                        g_t[:, :tw], op=OP.mult)
                nxt.append((t0, tw, ht, f_t, vb_t))
            for item in pending:
                _emit_scan(nc, hpool, auxt, carry, out, item)
            pending = nxt
        for item in pending:
            _emit_scan(nc, hpool, auxt, carry, out, item)
    nc.compile()
    return nc


def _get_nc():
    if "nc" not in _nc_cache:
        _nc_cache["nc"] = _build_nc()
    return _nc_cache["nc"]


def _g_host(x):
    # exp(log_g(x)) of the reference, computed directly in fp32
    return np.where(x >= 0, x + 0.5, 1.0 / (1.0 + np.exp(-np.minimum(x, 0))))


def _pack_dpairs(mat):
    """[T-or-D rows laid out d, cols] -> rows reindexed (kg, k, i): d = kg*256 + 2k + i.
    Input [D, N] -> output [128, 2kg, 2i, N] with out[k, kg, i] = in[kg*256+2k+i]."""
    m = mat.reshape(2, 128, 2, -1)          # [kg, k, i, N]
    return np.ascontiguousarray(m.transpose(1, 0, 2, 3))  # [k, kg, i, N]


def _run(inputs, trace=False):
    x = np.asarray(inputs["x"], np.float32)
    h_0 = np.asarray(inputs["h_0"], np.float32)
    W = {k: np.asarray(inputs[k], np.float32)
         for k in ("W_f", "W_i", "W_h")}
    for k in ("b_f", "b_i", "b_h"):
        assert (np.asarray(inputs[k]) == 0).all(), \
            "device program folds zero biases"

    g0 = _g_host(h_0[:, 0, :])  # [B, H]

    # --- x streams, packed once per sample ------------------------------
    xins = []
    for b in range(B):
        x8 = x[b].astype(NPF8)                       # [T, D]
        dx = (x[b] - x8.astype(np.float32)).astype(NPF8)
        # [2s, T, D] -> rows (k, s, kg, i) cols T
        s_td = np.stack([x8, dx], axis=0)            # [2, T, D]
        m = s_td.reshape(2, T, 2, 128, 2)            # [s, T, kg, k, i]
        m = m.transpose(3, 0, 2, 4, 1)               # [k, s, kg, i, T]
        m = np.ascontiguousarray(m).reshape(128, 8, T)
        m[127, 7, :] = NPF8(1.0)                     # bias slot: s=1,kg=1,i=1
        xins.append(m)

    in_maps = []
    for c in range(NCORES):
        b, hh = divmod(c, 2)
        hs = slice(hh * HS, (hh + 1) * HS)
        w8f = _pack_dpairs(W["W_f"][:, hs].astype(NPF8))   # [128,2,2,256]
        w8i = _pack_dpairs(W["W_i"][:, hs].astype(NPF8))
        whb = BETA * W["W_h"][:, hs]
        w8h = whb.astype(NPF8)
        dwh = (whb - w8h.astype(np.float32)).astype(NPF8)
        w8h_p = _pack_dpairs(w8h)
        dwh_p = _pack_dpairs(dwh)
        wdx_p = w8h_p.copy()
        wdx_p[127, 1, 1, :] = NPF8(0.5 * BETA)       # bias row (d=511)
        # wab rows: ((kg*2+g)*2+ht)*2+i ; cols m in [0,128)
        wab = np.zeros((128, 16, 128), NPF8)
        wcm = np.zeros((128, 24, 128), NPF8)
        for kg in range(2):
            for i in range(2):
                for ht in range(NHT):
                    mcols = slice(ht * 128, (ht + 1) * 128)
                    for g, wp in ((0, w8f), (1, w8i)):
                        wab[:, ((kg * 2 + g) * 2 + ht) * 2 + i, :] = \
                            wp[:, kg, i, mcols]
                    for role, wp in ((0, w8h_p), (1, wdx_p), (2, dwh_p)):
                        wcm[:, ((kg * 3 + role) * 2 + ht) * 2 + i, :] = \
                            wp[:, kg, i, mcols]
        auxa = np.ascontiguousarray(
            (BETA * g0[b, hs]).reshape(NHT, 128).T.astype(np.float32))
        in_maps.append({"xin": xins[b], "wab": wab, "wc": wcm, "aux": auxa})

    nc = _get_nc()
    res = run_bass_kernel_spmd(nc, in_maps, core_ids=list(range(NCORES)),
                               trace=trace)

    out = np.empty((B, T + 1, H), np.float32)
    out[:, 0, :] = g0
    inv_beta = np.float32(1.0 / BETA)
    for c in range(NCORES):
        b, hh = divmod(c, 2)
        o = np.asarray(res.results[c]["out"], np.float32)  # [128, NHT, T]
        # channel = hh*256 + ht*128 + p
        out[b, 1:, hh * HS:(hh + 1) * HS] = \
            (o.transpose(2, 1, 0).reshape(T, HS)) * inv_beta
    return out, res


def kernel(**inputs):
    out, _ = _run(inputs)
    return out
